# revision 1
# baseline (speedup 1.0000x reference)
"""Trainium2 (Bass/Tile) kernel for nn_MaxWeightGNN (gnn_message_passing).

    z = concat([xp, max(segment_max(xp[src], dst), xp)], 1) @ W.T,
    xp = prod(x, axis=1)

Strategy (8 NeuronCores, SPMD, one NEFF):
  * Nodes are sharded by dst range: core c owns nodes [c*32768, (c+1)*32768)
    and receives exactly the edges pointing into its range (edge-parallel by
    destination), so no cross-core reduction is needed.
  * The host precomputes the per-edge message xp[src] = x0*x1 (a gather, as
    the original layout already did per-edge) and lays it out as ONE fp16
    plane of padded, class-grouped, slot-major windows: each node's
    incoming-edge run is padded to a multiple of W=8 slots; nodes with the
    same number m of blocks are grouped so the device reduces them with
    uniform windows; tiny classes are merged upward to cut reduce op count.
  * Slot-major layout: window w's 8 slots live at columns s*NW + w, so the
    windowed max is a tree of contiguous tensor_tensor max ops (2x DVE mode
    for fp16) instead of the 1x-mode tensor_reduce; a per-class level-2
    reduce (m windows -> 1) then the self-loop max + [1,2] combine finish it.
  * The whole 4.6 MB per-core plane is double-buffered in SBUF: the timing
    loop software-pipelines across the For_i all-engine barrier - compute
    on buffer A fully overlaps the dual-HWDGE-queue stream into buffer B
    (node inputs ride the scalar HWDGE queue, z writeback the SWDGE queue
    with 528B+ lines - sub-512B SWDGE packets stall the SDMA round-robin).
    8x unroll amortizes the loop barrier.
  * Sentinel slots hold -65504 (fp16 lowest) so padding can never win a max;
    isolated nodes fall out of the self-loop max exactly like the
    reference's -inf semantics. fp16 quantization of the edge messages puts
    the end-to-end error at ~3e-4, far under the 2e-2 gate.

kernel(**inputs) takes the FULL inputs and returns the FULL [262144, 1]
float32 output; sharding/unsharding happens inside.
"""

import numpy as np

N_NODES = 262144
N_EDGES = 16777216
N_CORES = 8
P = 128
W = 8
SENT = np.float16(-65504.0)       # fp16 lowest: padding can never win a max
NEG_BIG = np.float16(-65504.0)


# ----------------------------------------------------------------------
# Host-side sharding/layout
# ----------------------------------------------------------------------

def build_layout(x, edge_index, n_cores=N_CORES):
    """Shard edges by dst range and build the per-core padded slot layout."""
    n = x.shape[0]
    npc = n // n_cores
    src = np.asarray(edge_index[0], dtype=np.int64)
    dst = np.asarray(edge_index[1], dtype=np.int64)
    order = np.argsort(dst, kind="stable")
    src_s = src[order]
    dst_s = dst[order]
    bounds = np.searchsorted(dst_s, np.arange(0, n + npc, npc))
    deg_all = np.bincount(dst_s, minlength=n)

    cores = []
    for c in range(n_cores):
        deg = deg_all[c * npc:(c + 1) * npc]
        blocks = (deg + W - 1) // W        # number of W-wide windows; 0 = isolated
        cores.append(dict(lo=bounds[c], hi=bounds[c + 1], deg=deg, blocks=blocks))

    # merge tiny classes upward (cascading) to cut level-2 reduce op count;
    # the slot cost is bounded by 256 nodes x gap x W per merge
    while True:
        cls = sorted(set(np.unique(np.concatenate([c["blocks"] for c in cores]))) - {0})
        merged = False
        for mi, m in enumerate(cls[:-1]):
            cnt = max(int((c["blocks"] == m).sum()) for c in cores)
            gap = cls[mi + 1] - m
            if cnt <= 256 and gap <= 2:
                for c in cores:
                    c["blocks"][c["blocks"] == m] = cls[mi + 1]
                merged = True
                break
        if not merged:
            break

    classes = sorted(set(np.unique(np.concatenate([c["blocks"] for c in cores]))) - {0})
    n0_max = max(int((c["blocks"] == 0).sum()) for c in cores)
    cols0 = (n0_max + P - 1) // P if n0_max > 0 else 0
    ncols_m = {}
    for m in classes:
        nm_max = max(int((c["blocks"] == m).sum()) for c in cores)
        ncols_m[m] = (nm_max + P - 1) // P
    # biggest window footprint first: heavy level-2 reduces complete early in
    # the stream, leaving only tiny classes (and their reduces) for the tail
    classes = sorted(classes, key=lambda m: -ncols_m[m] * m)

    NCOL = -(-(cols0 + sum(ncols_m.values())) // 8) * 8   # 528B+ DMA lines
    acc_off = {}
    woff = {}
    coff = cols0
    wtot = 0
    for m in classes:
        acc_off[m] = coff
        woff[m] = wtot
        coff += ncols_m[m]
        wtot += ncols_m[m] * m

    NW = -(-wtot // 16) * 16          # pad windows to a multiple of 16 (DVE align)
    TOT = NW * W
    nchunks = 1                       # whole plane is one tree (fully resident)
    VC = NW

    xp = (np.asarray(x[:, 0], dtype=np.float32)
          * np.asarray(x[:, 1], dtype=np.float32))
    xp16 = xp.astype(np.float16)
    max_m = max(classes) if classes else 0
    woff_arr = np.zeros(max_m + 1, dtype=np.int64)
    for m in classes:
        woff_arr[m] = woff[m]

    parts = []
    for c in range(n_cores):
        cc = cores[c]
        deg, blocks = cc["deg"], cc["blocks"]
        lo, hi = int(cc["lo"]), int(cc["hi"])
        e_src = src_s[lo:hi]
        e_dstl = dst_s[lo:hi] - c * npc
        run_start = np.zeros(npc, dtype=np.int64)
        run_start[1:] = np.cumsum(deg)[:-1]

        i_within = np.zeros(npc, dtype=np.int64)
        node_grid = np.full((NCOL, P), -1, dtype=np.int64)    # [col, p] -> local node
        nodes0 = np.flatnonzero(blocks == 0)
        if nodes0.size:
            gidx = np.arange(nodes0.size)
            node_grid[gidx // P, gidx % P] = nodes0
        for mi, m in enumerate(classes):
            nodes_m = np.flatnonzero(blocks == m)
            i_within[nodes_m] = np.arange(nodes_m.size)
            gidx = np.arange(nodes_m.size)
            node_grid[acc_off[m] + gidx // P, gidx % P] = nodes_m

        # per-edge flat slot address (slot-major within chunk)
        m_of_e = blocks[e_dstl]
        i_of_e = i_within[e_dstl]
        j_of_e = np.arange(len(e_src)) - run_start[e_dstl]     # rank in node's run
        wg = woff_arr[m_of_e] + (i_of_e // P) * m_of_e + j_of_e // W
        col = (wg // VC) * (VC * W) + (j_of_e % W) * VC + (wg % VC)
        flat = (i_of_e % P) * TOT + col

        plane = np.full(P * TOT, SENT, dtype=np.float16)
        plane[flat] = xp16[e_src]

        nxp = np.zeros((P, NCOL), dtype=np.float16)
        cols_v, p_v = np.nonzero(node_grid >= 0)
        nodes_v = node_grid[cols_v, p_v] + c * npc
        nxp[p_v, cols_v] = xp16[nodes_v]

        parts.append(dict(
            pairs=plane.reshape(P, TOT),
            npairs=nxp,
            node_grid=node_grid,
        ))

    meta = dict(TOT=TOT, NCOLF=NCOL, cols0=cols0, classes=classes, chunk=VC * W,
                ncols_m=ncols_m, woff=woff, acc_off=acc_off,
                nchunks=nchunks, NW=NW, npc=npc)
    return meta, parts


def build_inmaps(meta, parts, w):
    wb = np.repeat(np.asarray(w, dtype=np.float32).reshape(1, 2), P, axis=0)
    return [{"pairs": parts[c]["pairs"], "npairs": parts[c]["npairs"], "wb": wb}
            for c in range(len(parts))]


# ----------------------------------------------------------------------
# Device kernel (Bass/Tile)
# ----------------------------------------------------------------------

def build_kernel(meta, reps=1):
    import concourse.bacc as bacc
    import concourse.mybir as mybir
    import concourse.tile as tile

    TOT, NCOLF = meta["TOT"], meta["NCOLF"]

    nc = bacc.Bacc("TRN2", target_bir_lowering=False, debug=False,
                   num_devices=N_CORES)
    F16 = mybir.dt.float16
    pairs = nc.dram_tensor("pairs", [P, TOT], F16, kind="ExternalInput")
    npairs = nc.dram_tensor("npairs", [P, NCOLF], F16, kind="ExternalInput")
    wb = nc.dram_tensor("wb", [P, 2], mybir.dt.float32, kind="ExternalInput")
    zout = nc.dram_tensor("z", [P, NCOLF], F16, kind="ExternalOutput")

    UNROLL = 8
    with tile.TileContext(nc) as tc:
        with (
            tc.tile_pool(name="stream", bufs=1) as sp,
            tc.tile_pool(name="tree", bufs=2) as tp,
            tc.tile_pool(name="persist", bufs=1) as pp,
        ):
            # full double-buffer: the whole per-core plane fits in SBUF twice,
            # so iteration i computes on one buffer while the DMA queues fill
            # the other for iteration i+1 (software pipeline across the
            # For_i all-engine barrier)
            abA = sp.tile([P, TOT], F16, tag="abA")
            abB = sp.tile([P, TOT], F16, tag="abB")
            st = dict(
                l0=pp.tile([P, meta["NW"]], F16, tag="l0", name="l0"),
                acc=pp.tile([P, NCOLF], F16, tag="acc", name="acc"),
                nx=pp.tile([P, NCOLF], F16, tag="nx", name="nx"),
                w_t=pp.tile([P, 2], mybir.dt.float32, tag="w_t", name="w_t"),
                agg=pp.tile([P, NCOLF], F16, tag="agg", name="agg"),
                z=pp.tile([P, NCOLF], F16, tag="z", name="z"),
            )
            _emit_stream(nc, meta, abA, pairs, npairs, wb, st)   # prologue
            if reps == 1:
                _emit_compute(nc, meta, tp, abA, st, zout)
            else:
                def pair(x, y):
                    _emit_compute(nc, meta, tp, x, st, zout)
                    _emit_stream(nc, meta, y, pairs, npairs, wb, st)
                trips, rem = divmod(reps, UNROLL)
                if trips > 0:
                    with tc.For_i(0, trips, 1):
                        for _ in range(UNROLL // 2):
                            pair(abA, abB)
                            pair(abB, abA)
                for j in range(rem):
                    pair(abA, abB) if j % 2 == 0 else pair(abB, abA)
    return nc


def _emit_stream(nc, meta, ab, pairs, npairs, wb, st, nsplit=4):
    """Fill one stream buffer (both HWDGE queues) + node inputs (SWDGE)."""
    TOT = meta["TOT"]
    q = TOT // nsplit
    # node inputs first on the scalar HWDGE queue (no vector dependency, so
    # no head-of-line risk); keeps the small-descriptor traffic off SWDGE,
    # whose sub-512B packets stall the SDMA round-robin
    nc.scalar.dma_start(out=st["nx"][:], in_=npairs.ap())
    nc.scalar.dma_start(out=st["w_t"][:], in_=wb.ap())
    # nsplit plane slices alternating between the two HWDGE queues
    for i in range(nsplit):
        eng = nc.sync if i % 2 == 0 else nc.scalar
        eng.dma_start(out=ab[:, i * q:(i + 1) * q],
                      in_=pairs.ap()[:, i * q:(i + 1) * q])


def _emit_compute(nc, meta, tp, ab, st, zout):
    import concourse.mybir as mybir

    NCOLF, cols0 = meta["NCOLF"], meta["cols0"]
    classes, ncols_m = meta["classes"], meta["ncols_m"]
    woff, acc_off = meta["woff"], meta["acc_off"]
    C = meta["chunk"]
    VC = C // W
    nchunks = meta["nchunks"]
    NW = meta["NW"]
    F16 = mybir.dt.float16
    MAX = mybir.AluOpType.max
    l0, acc, nx, w_t = st["l0"], st["acc"], st["nx"], st["w_t"]
    agg, z = st["agg"], st["z"]

    if cols0 > 0:
        nc.vector.memset(acc[:, 0:cols0], NEG_BIG)

    cls_end = {m: woff[m] + ncols_m[m] * m for m in classes}

    def epilogue(lo, hi):
        # self-loop max + learned combine for acc cols [lo, hi), fp16 2x mode
        nc.vector.tensor_tensor(out=agg[:, lo:hi], in0=acc[:, lo:hi],
                                in1=nx[:, lo:hi], op=MAX)
        nc.vector.tensor_scalar_mul(z[:, lo:hi], agg[:, lo:hi], w_t[:, 1:2])
        nc.vector.scalar_tensor_tensor(
            out=z[:, lo:hi], in0=nx[:, lo:hi], scalar=w_t[:, 0:1],
            in1=z[:, lo:hi], op0=mybir.AluOpType.mult, op1=mybir.AluOpType.add,
        )
        nc.gpsimd.dma_start(out=zout.ap()[:, lo:hi], in_=z[:, lo:hi])

    for t in range(nchunks):
        vc = VC
        ct = vc * W
        cur, size, lvl = ab[:, t * C:(t + 1) * C], ct, 1
        while size > 2 * vc:
            nxt = tp.tile([P, size // 2], F16, tag=f"h{lvl}")
            nc.vector.tensor_tensor(out=nxt[:], in0=cur[:, :size // 2],
                                    in1=cur[:, size // 2:], op=MAX)
            cur, size, lvl = nxt[:], size // 2, lvl + 1
        nc.vector.tensor_tensor(out=l0[:, t * VC:t * VC + vc],
                                in0=cur[:, :vc], in1=cur[:, vc:], op=MAX)
        for m in classes:
            if t * VC < cls_end[m] <= t * VC + vc:
                r = ncols_m[m]
                nc.vector.reduce_max(
                    out=acc[:, acc_off[m]:acc_off[m] + r],
                    in_=l0[:, woff[m]:woff[m] + r * m].rearrange(
                        "p (c m) -> p c m", m=m),
                    axis=mybir.AxisListType.X,
                )
    epilogue(0, NCOLF)


# ----------------------------------------------------------------------
# SPMD execution (8 cores, one NEFF) via the bass2jax/PJRT path
# ----------------------------------------------------------------------

def build_runner(nc, n_cores=N_CORES):
    """Compile nc once; return run(in_maps) -> per-core output dicts."""
    import jax
    from jax.sharding import Mesh, PartitionSpec
    from jax.experimental.shard_map import shard_map
    from concourse import bass2jax
    from concourse.bass2jax import _bass_exec_p, partition_id_tensor
    import concourse.mybir as mybir

    bass2jax.install_neuronx_cc_hook()
    if not nc.is_finalized():
        nc.finalize()
    partition_name = nc.partition_id_tensor.name if nc.partition_id_tensor else None
    in_names, out_names, out_avals, zero_outs = [], [], [], []
    for alloc in nc.m.functions[0].allocations:
        if not isinstance(alloc, mybir.MemoryLocationSet):
            continue
        name = alloc.memorylocations[0].name
        if alloc.kind == "ExternalInput":
            if name != partition_name:
                in_names.append(name)
        elif alloc.kind == "ExternalOutput":
            shape = tuple(alloc.tensor_shape)
            dtype = mybir.dt.np(alloc.dtype)
            out_names.append(name)
            out_avals.append(jax.core.ShapedArray(shape, dtype))
            zero_outs.append(np.zeros(shape, dtype))
    n_params = len(in_names)
    n_outs = len(out_avals)
    all_in_names = in_names + out_names + ([partition_name] if partition_name else [])
    donate = tuple(range(n_params, n_params + n_outs))

    def _body(*args):
        operands = list(args)
        if partition_name is not None:
            operands.append(partition_id_tensor())
        outs = _bass_exec_p.bind(
            *operands, out_avals=tuple(out_avals), in_names=tuple(all_in_names),
            out_names=tuple(out_names), lowering_input_output_aliases=(),
            sim_require_finite=False, sim_require_nnan=False, nc=nc)
        return tuple(outs)

    devices = jax.devices()[:n_cores]
    mesh = Mesh(np.asarray(devices), ("core",))
    sharded = jax.jit(
        shard_map(_body, mesh=mesh,
                  in_specs=(PartitionSpec("core"),) * (n_params + n_outs),
                  out_specs=(PartitionSpec("core"),) * len(out_names),
                  check_rep=False),
        donate_argnums=donate, keep_unused=True)

    def run(in_maps):
        per_core = [[np.asarray(m[name]) for name in in_names] for m in in_maps]
        concat_in = [np.concatenate([per_core[c][i] for c in range(n_cores)], axis=0)
                     for i in range(n_params)]
        concat_zeros = [np.zeros((n_cores * z.shape[0], *z.shape[1:]), z.dtype)
                        for z in zero_outs]
        out_arrs = sharded(*concat_in, *concat_zeros)
        out_arrs = [np.asarray(a) for a in out_arrs]
        return [{name: out_arrs[i].reshape(n_cores, *out_avals[i].shape)[c]
                 for i, name in enumerate(out_names)} for c in range(n_cores)]

    return run


def assemble(meta, parts, results, n, n_cores=N_CORES):
    npc = meta["npc"]
    z_full = np.zeros((n, 1), dtype=np.float32)
    for c in range(n_cores):
        zc = results[c]["z"]
        ng = parts[c]["node_grid"]
        cols_v, p_v = np.nonzero(ng >= 0)
        z_full[ng[cols_v, p_v] + c * npc, 0] = zc[p_v, cols_v]
    return z_full


# ----------------------------------------------------------------------
# Entry point
# ----------------------------------------------------------------------

def kernel(x, edge_index, weights):
    x = np.asarray(x, dtype=np.float32)
    w = np.asarray(weights, dtype=np.float32)
    meta, parts = build_layout(x, edge_index, n_cores=N_CORES)
    in_maps = build_inmaps(meta, parts, w)
    last_err = None
    for _ in range(2):                    # one retry for transient device faults
        try:
            nc = build_kernel(meta)
            run = build_runner(nc)
            results = run(in_maps)
            return assemble(meta, parts, results, x.shape[0], n_cores=N_CORES)
        except Exception as e:            # noqa: BLE001
            last_err = e
    raise last_err



# revision 4
# speedup vs baseline: 1.1560x; 1.1560x over previous
"""Trainium2 (Bass/Tile) kernel for nn_MaxWeightGNN (gnn_message_passing).

    z = concat([xp, max(segment_max(xp[src], dst), xp)], 1) @ W.T,
    xp = prod(x, axis=1)

Strategy (8 NeuronCores, SPMD, one NEFF):
  * Nodes are sharded by dst range: core c owns nodes [c*32768, (c+1)*32768)
    and receives exactly the edges pointing into its range (edge-parallel by
    destination), so no cross-core reduction is needed.
  * The host precomputes the per-edge message xp[src] = x0*x1 (a gather, as
    the original layout already did per-edge) and lays it out as ONE fp16
    plane of padded, class-grouped, slot-major windows: each node's
    incoming-edge run is padded to a multiple of W=8 slots; nodes with the
    same number m of blocks are grouped so the device reduces them with
    uniform windows; tiny classes are merged upward to cut reduce op count.
  * Slot-major layout: window w's 8 slots live at columns s*NW + w, so the
    windowed max is a tree of contiguous tensor_tensor max ops (2x DVE mode
    for fp16) instead of the 1x-mode tensor_reduce; a per-class level-2
    reduce (m windows -> 1) then the self-loop max + [1,2] combine finish it.
  * The whole 4.6 MB per-core plane is double-buffered in SBUF: the timing
    loop software-pipelines across the For_i all-engine barrier - compute
    on buffer A fully overlaps the dual-HWDGE-queue stream into buffer B
    (node inputs ride the scalar HWDGE queue, z writeback the SWDGE queue
    with 528B+ lines - sub-512B SWDGE packets stall the SDMA round-robin).
    8x unroll amortizes the loop barrier.
  * Sentinel slots hold -65504 (fp16 lowest) so padding can never win a max;
    isolated nodes fall out of the self-loop max exactly like the
    reference's -inf semantics. fp16 quantization of the edge messages puts
    the end-to-end error at ~3e-4, far under the 2e-2 gate.

kernel(**inputs) takes the FULL inputs and returns the FULL [262144, 1]
float32 output; sharding/unsharding happens inside.
"""

import numpy as np

N_NODES = 262144
N_EDGES = 16777216
N_CORES = 8
P = 128
W = 8
SENT = np.float16(-65504.0)       # fp16 lowest: padding can never win a max
NEG_BIG = np.float16(-65504.0)


# ----------------------------------------------------------------------
# Host-side sharding/layout
# ----------------------------------------------------------------------

def build_layout(x, edge_index, n_cores=N_CORES):
    """Shard edges by dst range and build the per-core padded slot layout."""
    n = x.shape[0]
    npc = n // n_cores
    src = np.asarray(edge_index[0], dtype=np.int64)
    dst = np.asarray(edge_index[1], dtype=np.int64)
    order = np.argsort(dst, kind="stable")
    src_s = src[order]
    dst_s = dst[order]
    bounds = np.searchsorted(dst_s, np.arange(0, n + npc, npc))
    deg_all = np.bincount(dst_s, minlength=n)

    cores = []
    for c in range(n_cores):
        deg = deg_all[c * npc:(c + 1) * npc]
        blocks = (deg + W - 1) // W        # number of W-wide windows; 0 = isolated
        cores.append(dict(lo=bounds[c], hi=bounds[c + 1], deg=deg, blocks=blocks))

    # merge tiny classes upward (cascading) to cut level-2 reduce op count;
    # the slot cost is bounded by 256 nodes x gap x W per merge
    while True:
        cls = sorted(set(np.unique(np.concatenate([c["blocks"] for c in cores]))) - {0})
        merged = False
        for mi, m in enumerate(cls[:-1]):
            cnt = max(int((c["blocks"] == m).sum()) for c in cores)
            gap = cls[mi + 1] - m
            if cnt <= 256 and gap <= 2:
                for c in cores:
                    c["blocks"][c["blocks"] == m] = cls[mi + 1]
                merged = True
                break
        if not merged:
            break

    classes = sorted(set(np.unique(np.concatenate([c["blocks"] for c in cores]))) - {0})
    n0_max = max(int((c["blocks"] == 0).sum()) for c in cores)
    cols0 = (n0_max + P - 1) // P if n0_max > 0 else 0
    ncols_m = {}
    for m in classes:
        nm_max = max(int((c["blocks"] == m).sum()) for c in cores)
        ncols_m[m] = (nm_max + P - 1) // P
    # biggest window footprint first: heavy level-2 reduces complete early in
    # the stream, leaving only tiny classes (and their reduces) for the tail
    classes = sorted(classes, key=lambda m: -ncols_m[m] * m)

    NCOL = -(-(cols0 + sum(ncols_m.values())) // 8) * 8   # 528B+ DMA lines
    acc_off = {}
    woff = {}
    coff = cols0
    wtot = 0
    for m in classes:
        acc_off[m] = coff
        woff[m] = wtot
        coff += ncols_m[m]
        wtot += ncols_m[m] * m

    NW = -(-wtot // 16) * 16          # pad windows to a multiple of 16 (DVE align)
    TOT = NW * W
    nchunks = 1                       # whole plane is one tree (fully resident)
    VC = NW

    xp = (np.asarray(x[:, 0], dtype=np.float32)
          * np.asarray(x[:, 1], dtype=np.float32))
    xp16 = xp.astype(np.float16)
    max_m = max(classes) if classes else 0
    woff_arr = np.zeros(max_m + 1, dtype=np.int64)
    for m in classes:
        woff_arr[m] = woff[m]

    parts = []
    for c in range(n_cores):
        cc = cores[c]
        deg, blocks = cc["deg"], cc["blocks"]
        lo, hi = int(cc["lo"]), int(cc["hi"])
        e_src = src_s[lo:hi]
        e_dstl = dst_s[lo:hi] - c * npc
        run_start = np.zeros(npc, dtype=np.int64)
        run_start[1:] = np.cumsum(deg)[:-1]

        i_within = np.zeros(npc, dtype=np.int64)
        node_grid = np.full((NCOL, P), -1, dtype=np.int64)    # [col, p] -> local node
        nodes0 = np.flatnonzero(blocks == 0)
        if nodes0.size:
            gidx = np.arange(nodes0.size)
            node_grid[gidx // P, gidx % P] = nodes0
        for mi, m in enumerate(classes):
            nodes_m = np.flatnonzero(blocks == m)
            i_within[nodes_m] = np.arange(nodes_m.size)
            gidx = np.arange(nodes_m.size)
            node_grid[acc_off[m] + gidx // P, gidx % P] = nodes_m

        # per-edge flat slot address (slot-major within chunk)
        m_of_e = blocks[e_dstl]
        i_of_e = i_within[e_dstl]
        j_of_e = np.arange(len(e_src)) - run_start[e_dstl]     # rank in node's run
        wg = woff_arr[m_of_e] + (i_of_e // P) * m_of_e + j_of_e // W
        col = (wg // VC) * (VC * W) + (j_of_e % W) * VC + (wg % VC)
        flat = (i_of_e % P) * TOT + col

        plane = np.full(P * TOT, SENT, dtype=np.float16)
        plane[flat] = xp16[e_src]

        nxp = np.zeros((P, NCOL), dtype=np.float16)
        cols_v, p_v = np.nonzero(node_grid >= 0)
        nodes_v = node_grid[cols_v, p_v] + c * npc
        nxp[p_v, cols_v] = xp16[nodes_v]

        parts.append(dict(
            pairs=plane.reshape(P, TOT),
            npairs=nxp,
            node_grid=node_grid,
        ))

    meta = dict(TOT=TOT, NCOLF=NCOL, cols0=cols0, classes=classes, chunk=VC * W,
                ncols_m=ncols_m, woff=woff, acc_off=acc_off,
                nchunks=nchunks, NW=NW, npc=npc)
    return meta, parts


def build_inmaps(meta, parts, w):
    wb = np.repeat(np.asarray(w, dtype=np.float32).reshape(1, 2), P, axis=0)
    return [{"pairs": parts[c]["pairs"], "npairs": parts[c]["npairs"], "wb": wb}
            for c in range(len(parts))]


# ----------------------------------------------------------------------
# Device kernel (Bass/Tile)
# ----------------------------------------------------------------------

def build_kernel(meta, reps=1):
    import concourse.bacc as bacc
    import concourse.mybir as mybir
    import concourse.tile as tile

    TOT, NCOLF = meta["TOT"], meta["NCOLF"]

    nc = bacc.Bacc("TRN2", target_bir_lowering=False, debug=False,
                   num_devices=N_CORES)
    F16 = mybir.dt.float16
    pairs = nc.dram_tensor("pairs", [P, TOT], F16, kind="ExternalInput")
    npairs = nc.dram_tensor("npairs", [P, NCOLF], F16, kind="ExternalInput")
    wb = nc.dram_tensor("wb", [P, 2], mybir.dt.float32, kind="ExternalInput")
    zout = nc.dram_tensor("z", [P, NCOLF], F16, kind="ExternalOutput")

    UNROLL = 8
    with tile.TileContext(nc) as tc:
        with (
            tc.tile_pool(name="stream", bufs=1) as sp,
            tc.tile_pool(name="tree", bufs=2) as tp,
            tc.tile_pool(name="persist", bufs=1) as pp,
        ):
            # full double-buffer: the whole per-core plane fits in SBUF twice,
            # so iteration i computes on one buffer while the DMA queues fill
            # the other for iteration i+1 (software pipeline across the
            # For_i all-engine barrier).  nx/w_t/z are double-buffered too so
            # their WAR hazard against the previous iteration's epilogue can
            # never stall the HWDGE queues that carry the big plane slices.
            abA = sp.tile([P, TOT], F16, tag="abA")
            abB = sp.tile([P, TOT], F16, tag="abB")
            st = dict(
                l0=pp.tile([P, meta["NW"]], F16, tag="l0", name="l0"),
                acc=pp.tile([P, NCOLF], F16, tag="acc", name="acc"),
                nx=[pp.tile([P, NCOLF], F16, tag="nxA", name="nxA"),
                    pp.tile([P, NCOLF], F16, tag="nxB", name="nxB")],
                w_t=[pp.tile([P, 2], mybir.dt.float32, tag="wtA", name="wtA"),
                     pp.tile([P, 2], mybir.dt.float32, tag="wtB", name="wtB")],
                agg=pp.tile([P, NCOLF], F16, tag="agg", name="agg"),
                z=[pp.tile([P, NCOLF], F16, tag="zA", name="zA"),
                   pp.tile([P, NCOLF], F16, tag="zB", name="zB")],
            )
            _emit_stream(nc, meta, abA, pairs, npairs, wb, st, 0)  # prologue
            if reps == 1:
                _emit_compute(nc, meta, tp, abA, st, zout, 0)
            else:
                def pair(x, y, i):
                    _emit_compute(nc, meta, tp, x, st, zout, i)
                    _emit_stream(nc, meta, y, pairs, npairs, wb, st, 1 - i)
                trips, rem = divmod(reps, UNROLL)
                if trips > 0:
                    with tc.For_i(0, trips, 1):
                        for _ in range(UNROLL // 2):
                            pair(abA, abB, 0)
                            pair(abB, abA, 1)
                for j in range(rem):
                    pair(abA, abB, 0) if j % 2 == 0 else pair(abB, abA, 1)
    return nc


def _emit_stream(nc, meta, ab, pairs, npairs, wb, st, idx, nsplit=2):
    """Fill one stream buffer (both HWDGE queues) + node inputs.

    Plane slices go FIRST on both queues: they are the bulk of the traffic
    and their WAR hazard (level-1 read of the same buffer, one iteration
    back) clears early in the previous compute.  nx/w_t go LAST — they are
    only needed by the epilogue and are double-buffered, so they never
    block the plane stream.
    """
    TOT = meta["TOT"]
    q = TOT // nsplit
    for i in range(nsplit):
        eng = nc.sync if i % 2 == 0 else nc.scalar
        eng.dma_start(out=ab[:, i * q:(i + 1) * q],
                      in_=pairs.ap()[:, i * q:(i + 1) * q])
    nc.scalar.dma_start(out=st["nx"][idx][:], in_=npairs.ap())
    nc.sync.dma_start(out=st["w_t"][idx][:], in_=wb.ap())


def _emit_compute(nc, meta, tp, ab, st, zout, idx):
    import concourse.mybir as mybir

    NCOLF, cols0 = meta["NCOLF"], meta["cols0"]
    classes, ncols_m = meta["classes"], meta["ncols_m"]
    woff, acc_off = meta["woff"], meta["acc_off"]
    C = meta["chunk"]
    VC = C // W
    nchunks = meta["nchunks"]
    NW = meta["NW"]
    F16 = mybir.dt.float16
    MAX = mybir.AluOpType.max
    l0, acc, nx, w_t = st["l0"], st["acc"], st["nx"][idx], st["w_t"][idx]
    agg, z = st["agg"], st["z"][idx]

    if cols0 > 0:
        nc.vector.memset(acc[:, 0:cols0], NEG_BIG)

    cls_end = {m: woff[m] + ncols_m[m] * m for m in classes}

    def epilogue(lo, hi):
        # self-loop max + learned combine for acc cols [lo, hi), fp16 2x mode
        nc.vector.tensor_tensor(out=agg[:, lo:hi], in0=acc[:, lo:hi],
                                in1=nx[:, lo:hi], op=MAX)
        nc.vector.tensor_scalar_mul(z[:, lo:hi], agg[:, lo:hi], w_t[:, 1:2])
        nc.vector.scalar_tensor_tensor(
            out=z[:, lo:hi], in0=nx[:, lo:hi], scalar=w_t[:, 0:1],
            in1=z[:, lo:hi], op0=mybir.AluOpType.mult, op1=mybir.AluOpType.add,
        )
        nc.gpsimd.dma_start(out=zout.ap()[:, lo:hi], in_=z[:, lo:hi])

    for t in range(nchunks):
        vc = VC
        ct = vc * W
        cur, size, lvl = ab[:, t * C:(t + 1) * C], ct, 1
        while size > 2 * vc:
            nxt = tp.tile([P, size // 2], F16, tag=f"h{lvl}")
            nc.vector.tensor_tensor(out=nxt[:], in0=cur[:, :size // 2],
                                    in1=cur[:, size // 2:], op=MAX)
            cur, size, lvl = nxt[:], size // 2, lvl + 1
        nc.vector.tensor_tensor(out=l0[:, t * VC:t * VC + vc],
                                in0=cur[:, :vc], in1=cur[:, vc:], op=MAX)
        for m in classes:
            if t * VC < cls_end[m] <= t * VC + vc:
                r = ncols_m[m]
                nc.vector.reduce_max(
                    out=acc[:, acc_off[m]:acc_off[m] + r],
                    in_=l0[:, woff[m]:woff[m] + r * m].rearrange(
                        "p (c m) -> p c m", m=m),
                    axis=mybir.AxisListType.X,
                )
    epilogue(0, NCOLF)


# ----------------------------------------------------------------------
# SPMD execution (8 cores, one NEFF) via the bass2jax/PJRT path
# ----------------------------------------------------------------------

def build_runner(nc, n_cores=N_CORES):
    """Compile nc once; return run(in_maps) -> per-core output dicts."""
    import jax
    from jax.sharding import Mesh, PartitionSpec
    from jax.experimental.shard_map import shard_map
    from concourse import bass2jax
    from concourse.bass2jax import _bass_exec_p, partition_id_tensor
    import concourse.mybir as mybir

    bass2jax.install_neuronx_cc_hook()
    if not nc.is_finalized():
        nc.finalize()
    partition_name = nc.partition_id_tensor.name if nc.partition_id_tensor else None
    in_names, out_names, out_avals, zero_outs = [], [], [], []
    for alloc in nc.m.functions[0].allocations:
        if not isinstance(alloc, mybir.MemoryLocationSet):
            continue
        name = alloc.memorylocations[0].name
        if alloc.kind == "ExternalInput":
            if name != partition_name:
                in_names.append(name)
        elif alloc.kind == "ExternalOutput":
            shape = tuple(alloc.tensor_shape)
            dtype = mybir.dt.np(alloc.dtype)
            out_names.append(name)
            out_avals.append(jax.core.ShapedArray(shape, dtype))
            zero_outs.append(np.zeros(shape, dtype))
    n_params = len(in_names)
    n_outs = len(out_avals)
    all_in_names = in_names + out_names + ([partition_name] if partition_name else [])
    donate = tuple(range(n_params, n_params + n_outs))

    def _body(*args):
        operands = list(args)
        if partition_name is not None:
            operands.append(partition_id_tensor())
        outs = _bass_exec_p.bind(
            *operands, out_avals=tuple(out_avals), in_names=tuple(all_in_names),
            out_names=tuple(out_names), lowering_input_output_aliases=(),
            sim_require_finite=False, sim_require_nnan=False, nc=nc)
        return tuple(outs)

    devices = jax.devices()[:n_cores]
    mesh = Mesh(np.asarray(devices), ("core",))
    sharded = jax.jit(
        shard_map(_body, mesh=mesh,
                  in_specs=(PartitionSpec("core"),) * (n_params + n_outs),
                  out_specs=(PartitionSpec("core"),) * len(out_names),
                  check_rep=False),
        donate_argnums=donate, keep_unused=True)

    def run(in_maps):
        per_core = [[np.asarray(m[name]) for name in in_names] for m in in_maps]
        concat_in = [np.concatenate([per_core[c][i] for c in range(n_cores)], axis=0)
                     for i in range(n_params)]
        concat_zeros = [np.zeros((n_cores * z.shape[0], *z.shape[1:]), z.dtype)
                        for z in zero_outs]
        out_arrs = sharded(*concat_in, *concat_zeros)
        out_arrs = [np.asarray(a) for a in out_arrs]
        return [{name: out_arrs[i].reshape(n_cores, *out_avals[i].shape)[c]
                 for i, name in enumerate(out_names)} for c in range(n_cores)]

    return run


def assemble(meta, parts, results, n, n_cores=N_CORES):
    npc = meta["npc"]
    z_full = np.zeros((n, 1), dtype=np.float32)
    for c in range(n_cores):
        zc = results[c]["z"]
        ng = parts[c]["node_grid"]
        cols_v, p_v = np.nonzero(ng >= 0)
        z_full[ng[cols_v, p_v] + c * npc, 0] = zc[p_v, cols_v]
    return z_full


# ----------------------------------------------------------------------
# Entry point
# ----------------------------------------------------------------------

def kernel(x, edge_index, weights):
    x = np.asarray(x, dtype=np.float32)
    w = np.asarray(weights, dtype=np.float32)
    meta, parts = build_layout(x, edge_index, n_cores=N_CORES)
    in_maps = build_inmaps(meta, parts, w)
    last_err = None
    for _ in range(2):                    # one retry for transient device faults
        try:
            nc = build_kernel(meta)
            run = build_runner(nc)
            results = run(in_maps)
            return assemble(meta, parts, results, x.shape[0], n_cores=N_CORES)
        except Exception as e:            # noqa: BLE001
            last_err = e
    raise last_err

